# revision 29
# baseline (speedup 1.0000x reference)
"""Trainium2 Bass kernel for BertAdapterCapsuleMask — fp8 DoubleRow version.

Self-contained: takes full (unsharded) numpy inputs, shards across 8
NeuronCores, runs a fused Bass/Tile kernel per core, gathers the full output.

Key semantics note: the reference's `h_caps = vote.reshape(B, S, M*C)` is an
m-major flat reinterpret, so token n's 9 capsule inputs are vote values of
tokens ~3n from a single m-block — NOT batch-local.  We handle this by
computing the cheap part (semantic capsules -> squash -> routing priors,
~0.5% of FLOPs) exactly on the host, pre-scrambling priors into each core's
consumer "stream order" (rows (d, r, c), d = which-of-3-source-tokens), and
running the iterative routing + all heavy matmuls on device.  In stream
order the final vote tile IS h_caps in consumer layout, so the larger/adapter
matmuls consume it directly.

Perf design: the two adapter matmuls (H->A, A->H over 2048 tokens/core)
dominate PE time.  They run in fp8e4m3 with MatmulPerfMode.DoubleRow (two
128-deep K-subtiles per instruction at 0.5 cyc/row = 4x f32r row throughput,
2x fewer PE instructions).  Weights are pre-scaled (x64 / x128) on the host
to center their tiny magnitudes in fp8 range; the scale is undone for free
in the gelu activation's input-scale.  x streams in as bf16 (only feeds the
fp8 adapter input), the device returns the pre-gate second gelu in bf16, and
the host applies the (exact) gfc2 gate and adds the f32 skip connection.
Measured end-to-end max rel err ~5e-3 (gate: 2e-2).
"""

import sys

sys.path.insert(0, "/opt/trn_rl_repo")
import numpy as np

B, S, H, A, T, C, M3 = 128, 128, 768, 2000, 10, 3, 3
NCORES = 8
NTOK = B * S                  # 16384 tokens total
NCT = NTOK // NCORES          # 2048 tokens per core
NCHUNK = 512                  # tokens per pipeline chunk (PSUM bank = 512 f32)
NCH = NCT // NCHUNK           # 4 chunks per core
APAD = 2048                   # A=2000 zero-padded to 16x128
AC = APAD // 128              # 16 a-chunks
HC = H // 128                 # 6 h-chunks
KP1 = HC // 2                 # 3 DoubleRow k-pairs for mm1 (K=H)
KP2 = AC // 2                 # 8 DoubleRow k-pairs for mm2 (K=A)
EPS = 1e-16
NV = M3 * C                   # 9 rows: (d, c)
S1 = 64.0                     # fp8 pre-scale on w1
S2 = 128.0                    # fp8 pre-scale on w2

_CACHE = {}


def _sel_shapes(Teff):
    NL = M3 * Teff
    NP = M3 * Teff * C
    return {
        "sq9to3": (NV, M3),      # sum squares of vote per d
        "exp3to9": (M3, NV),     # per-d scalar -> (d, c)
        "exp9toNP": (NV, NP),    # outputs (d,c) -> (d, r, c)
        "redNPtoNL": (NP, NL),   # sum over c: (d,r,c) -> (d,r)
        "expNLtoNP": (NL, NP),   # E (d,r) -> (d,r,c)
        "redNLto3": (NL, M3),    # sum over r: (d,r) -> d
        "redNPto9": (NP, NV),    # sum over r: (d,r,c) -> (d,c)
    }


def _build(Teff, repeat=1, loop_repeat=1, psum_mm=3, psum_rt=4,
           no_io_dma=False, weights_outside=False):
    """Build + compile the per-core Bass program (shapes depend on Teff=t+1).

    repeat>1 unrolls the whole computation R times (timing builds only)."""
    import concourse.bacc as bacc
    import concourse.mybir as mybir
    import concourse.tile as tile

    f32 = mybir.dt.float32
    f32r = mybir.dt.float32r
    bf16 = mybir.dt.bfloat16
    f8 = mybir.dt.float8e4
    i32 = mybir.dt.int32
    DR = mybir.MatmulPerfMode.DoubleRow
    AF = mybir.ActivationFunctionType
    OP = mybir.AluOpType
    # Schraudolph exp constants: exp(x) ~= bitcast_f32(round(EXP_A*x + EXP_B)),
    # max rel err 2.98% over x in [-30, 8] (bits values are exact multiples of
    # the f32 ulp at ~1e9, so round-vs-trunc convert semantics agree).
    EXP_A, EXP_B = 12102203.0, 1064987000.0
    RSQ_B = 1597463007.0      # float-domain fast-inverse-sqrt magic

    NL = M3 * Teff
    NP = M3 * Teff * C
    sel_shapes = _sel_shapes(Teff)

    nc = bacc.Bacc("TRN2", target_bir_lowering=False, debug=False)

    dx = nc.dram_tensor("xT", [HC, 128, NCT], bf16, kind="ExternalInput").ap()
    dw1 = nc.dram_tensor("w1p", [128, HC, APAD], f8, kind="ExternalInput").ap()
    dw2 = nc.dram_tensor("w2p", [128, AC, H], f8, kind="ExternalInput").ap()
    dlw9 = nc.dram_tensor("lw9", [NV, H], f32r, kind="ExternalInput").ap()
    dp54 = nc.dram_tensor("p54s", [NP, NCT], f32, kind="ExternalInput").ap()
    do0 = nc.dram_tensor("o0s", [NV, NCT], f32r, kind="ExternalInput").ap()
    dcon = nc.dram_tensor("consts", [128, 35], f32, kind="ExternalInput").ap()
    dsel = {
        k: nc.dram_tensor(k, list(v), f32r, kind="ExternalInput").ap()
        for k, v in sel_shapes.items()
    }
    dout = nc.dram_tensor("outT", [HC, 128, NCT], bf16, kind="ExternalOutput").ap()

    with tile.TileContext(nc) as tc, \
         nc.allow_low_precision(reason="fp8/bf16 tiles feed PE matmuls by design"):
        with tc.tile_pool(name="wp", bufs=1) as wp, \
             tc.tile_pool(name="px", bufs=1) as px, \
             tc.tile_pool(name="pout", bufs=1) as pout, \
             tc.tile_pool(name="ph1", bufs=1) as ph1, \
             tc.tile_pool(name="phT", bufs=1) as phT, \
             tc.tile_pool(name="prt", bufs=12) as prt, \
             tc.tile_pool(name="pp54", bufs=1) as pp54, \
             tc.tile_pool(name="po0", bufs=1) as po0, \
             tc.tile_pool(name="pL", bufs=2) as pL, \
             tc.tile_pool(name="psmm", bufs=psum_mm, space="PSUM") as psmm, \
             tc.tile_pool(name="psrt", bufs=psum_rt, space="PSUM") as psrt:

            # ---- small constant loads (selectors, consts, lw9) ------------
            selt = {}
            for k, (pp, mm) in sel_shapes.items():
                tl = wp.tile([pp, mm], f32r, name=f"sel_{k}")
                nc.sync.dma_start(tl[:], dsel[k][:, :])
                selt[k] = tl
            cont = wp.tile([128, 35], f32, name="consts")
            nc.sync.dma_start(cont[:], dcon[:, :])
            lw9t = wp.tile([NV, H], f32r, name="lw9")
            nc.sync.dma_start(lw9t[:], dlw9[:, :])
            w1t = wp.tile([128, HC, APAD], f8, name="w1p")
            w2t = wp.tile([128, AC, H], f8, name="w2p")

            b1 = lambda a: cont[:, a:a + 1]            # noqa: E731
            b2 = lambda h: cont[:, 16 + h:17 + h]      # noqa: E731
            lb = lambda h: cont[:, 28 + h:29 + h]      # noqa: E731
            epsc = lambda n: cont[0:n, 34:35]          # noqa: E731

            def coef_chain(nm, sq_ps, ngrp):
                """squash coefficient from group sum-of-squares psum [ngrp,n]:
                coef = s / ((1+s) * sqrt(s)),  s = sq+eps,  f32r tile.

                sqrt via float-domain fast-inverse-sqrt (magic constant + 2
                Newton steps) on the otherwise-idle Pool engine, keeping the
                ACT engine pure-Gelu (no activation-table thrash)."""
                scp = prt.tile([ngrp, NCHUNK], f32, tag="rt", name=f"scp_{nm}")
                nc.vector.tensor_scalar_add(scp[:], sq_ps[:], EPS)
                bf = prt.tile([ngrp, NCHUNK], f32, tag="rt", name=f"bf_{nm}")
                nc.gpsimd.tensor_copy(bf[:], scp[:].bitcast(i32))
                yf = prt.tile([ngrp, NCHUNK], f32, tag="rt", name=f"yf_{nm}")
                nc.gpsimd.tensor_scalar(yf[:], bf[:], -0.5, RSQ_B, OP.mult, OP.add)
                y = prt.tile([ngrp, NCHUNK], f32, tag="rt", name=f"y0_{nm}")
                nc.gpsimd.tensor_copy(y[:].bitcast(i32), yf[:])
                for it in range(2):
                    u = prt.tile([ngrp, NCHUNK], f32, tag="rt", name=f"u{it}_{nm}")
                    nc.gpsimd.tensor_mul(u[:], y[:], y[:])
                    w = prt.tile([ngrp, NCHUNK], f32, tag="rt", name=f"w{it}_{nm}")
                    nc.gpsimd.tensor_mul(w[:], u[:], scp[:])
                    v = prt.tile([ngrp, NCHUNK], f32, tag="rt", name=f"v{it}_{nm}")
                    nc.gpsimd.tensor_scalar(v[:], w[:], -0.5, 1.5, OP.mult, OP.add)
                    y2 = prt.tile([ngrp, NCHUNK], f32, tag="rt", name=f"y{it + 1}_{nm}")
                    nc.gpsimd.tensor_mul(y2[:], y[:], v[:])
                    y = y2
                sqr = prt.tile([ngrp, NCHUNK], f32, tag="rt", name=f"sqr_{nm}")
                nc.gpsimd.tensor_mul(sqr[:], y[:], scp[:])
                sp1 = prt.tile([ngrp, NCHUNK], f32, tag="rt", name=f"sp1_{nm}")
                nc.gpsimd.tensor_scalar_add(sp1[:], scp[:], 1.0)
                den = prt.tile([ngrp, NCHUNK], f32, tag="rt", name=f"den_{nm}")
                nc.gpsimd.tensor_mul(den[:], sp1[:], sqr[:])
                rec = prt.tile([ngrp, NCHUNK], f32, tag="rt", name=f"rec_{nm}")
                nc.vector.reciprocal(rec[:], den[:])
                coef = prt.tile([ngrp, NCHUNK], f32r, tag="rt", name=f"coef_{nm}")
                nc.gpsimd.tensor_mul(coef[:], scp[:], rec[:])
                return coef

            def exp_pool(nm, L_tile, npart):
                """E = exp(L) via Schraudolph bit-trick, on the Pool engine."""
                ebf = prt.tile([npart, NCHUNK], f32, tag="rt", name=f"ebf_{nm}")
                nc.gpsimd.tensor_scalar(ebf[:], L_tile[:], EXP_A, EXP_B,
                                        OP.mult, OP.add)
                ebi = prt.tile([npart, NCHUNK], i32, tag="rt", name=f"ebi_{nm}")
                nc.gpsimd.tensor_copy(ebi[:], ebf[:])
                # matmul-consumed tiles must be written with f32r rounding
                E = prt.tile([npart, NCHUNK], f32r, tag="rt", name=f"E_{nm}")
                nc.gpsimd.tensor_copy(E[:], ebi[:].bitcast(f32))
                return E

            state = {}

            def routing_units(nm, c0, t):
                """Routing chain for chunk c0, one yield per PE-anchored unit.

                Stores vt2 (h_caps tile) in state[nm]; t holds the
                per-iteration full tiles (p54f, o0f, xf, of)."""
                cs = c0 * NCHUNK
                p54 = t["p54f"][:, cs:cs + NCHUNK]
                o0sl = t["o0f"][:, cs:cs + NCHUNK]
                yield

                def squash9_units(snm, vote_src):
                    vv = prt.tile([NV, NCHUNK], f32r, tag="rt", name=f"vv_{snm}")
                    nc.vector.tensor_mul(vv[:], vote_src[:], vote_src[:])
                    yield
                    sqm = psrt.tile([M3, NCHUNK], f32, tag="ps_rt", name=f"sqm_{snm}")
                    nc.tensor.matmul(sqm[:], selt["sq9to3"][:], vv[:], start=True, stop=True)
                    coef = coef_chain(snm, sqm, M3)
                    yield
                    ce9 = psrt.tile([NV, NCHUNK], f32, tag="ps_rt", name=f"ce9_{snm}")
                    nc.tensor.matmul(ce9[:], selt["exp3to9"][:], coef[:], start=True, stop=True)
                    outp = prt.tile([NV, NCHUNK], f32r, tag="rt", name=f"outp_{snm}")
                    nc.vector.tensor_mul(outp[:], vote_src[:], ce9[:])
                    state[f"outp_{snm}"] = outp

                def delta_units(snm, outp):
                    o54 = psrt.tile([NP, NCHUNK], f32, tag="ps_rt", name=f"o54_{snm}")
                    nc.tensor.matmul(o54[:], selt["exp9toNP"][:], outp, start=True, stop=True)
                    prd = prt.tile([NP, NCHUNK], f32r, tag="rt", name=f"prd_{snm}")
                    nc.vector.tensor_mul(prd[:], p54, o54[:])
                    yield
                    dl = psrt.tile([NL, NCHUNK], f32, tag="ps_rt", name=f"dl_{snm}")
                    nc.tensor.matmul(dl[:], selt["redNPtoNL"][:], prd[:], start=True, stop=True)
                    state[f"dl_{snm}"] = dl

                def vote_units(snm, e_tile):
                    dn = psrt.tile([M3, NCHUNK], f32, tag="ps_rt", name=f"dn_{snm}")
                    nc.tensor.matmul(dn[:], selt["redNLto3"][:], e_tile[:], start=True, stop=True)
                    rcd = prt.tile([M3, NCHUNK], f32r, tag="rt", name=f"rcd_{snm}")
                    nc.vector.reciprocal(rcd[:], dn[:])
                    yield
                    e54 = psrt.tile([NP, NCHUNK], f32, tag="ps_rt", name=f"e54_{snm}")
                    nc.tensor.matmul(e54[:], selt["expNLtoNP"][:], e_tile[:], start=True, stop=True)
                    pre = prt.tile([NP, NCHUNK], f32r, tag="rt", name=f"pre_{snm}")
                    nc.vector.tensor_mul(pre[:], p54, e54[:])
                    yield
                    vu = psrt.tile([NV, NCHUNK], f32, tag="ps_rt", name=f"vu_{snm}")
                    nc.tensor.matmul(vu[:], selt["redNPto9"][:], pre[:], start=True, stop=True)
                    vusb = prt.tile([NV, NCHUNK], f32, tag="rt", name=f"vusb_{snm}")
                    nc.vector.tensor_copy(vusb[:], vu[:])
                    yield
                    r9 = psrt.tile([NV, NCHUNK], f32, tag="ps_rt", name=f"r9_{snm}")
                    nc.tensor.matmul(r9[:], selt["exp3to9"][:], rcd[:], start=True, stop=True)
                    vt = prt.tile([NV, NCHUNK], f32r, tag="rt", name=f"vt_{snm}")
                    nc.vector.tensor_mul(vt[:], vusb[:], r9[:])
                    state[f"vt_{snm}"] = vt

                # iter 0: outputs0 = squash(mean-priors) precomputed on host
                yield from delta_units(f"{nm}_0", o0sl)
                yield
                L1 = pL.tile([NL, NCHUNK], f32, tag="L", name=f"L1_{nm}")
                nc.vector.tensor_copy(L1[:], state[f"dl_{nm}_0"][:])
                E1 = exp_pool(f"E1_{nm}", L1, NL)
                # iter 1
                yield from vote_units(f"{nm}_1", E1)
                yield
                vt1 = state[f"vt_{nm}_1"]
                yield from squash9_units(f"{nm}_1s", vt1)
                yield
                yield from delta_units(f"{nm}_1", state[f"outp_{nm}_1s"])
                yield
                L2 = pL.tile([NL, NCHUNK], f32, tag="L", name=f"L2_{nm}")
                nc.vector.tensor_add(L2[:], L1[:], state[f"dl_{nm}_1"][:])
                E2 = exp_pool(f"E2_{nm}", L2, NL)
                # iter 2 (final)
                yield from vote_units(f"{nm}_2", E2)
                state[f"vt2_{nm}"] = state[f"vt_{nm}_2"]

            def big_units(nm, c0, t):
                """larger + adapter matmuls for chunk c0, one yield per psum group."""
                cs = c0 * NCHUNK
                vt2 = state[f"vt2_{nm}"]
                xf = t["xf"]
                hTp = phT.tile([128, HC, NCHUNK], f8, tag="hTp", name=f"hTp_{nm}")
                for h in range(HC):
                    pl = psmm.tile([128, NCHUNK], f32, tag="mm", name=f"pl_{nm}_{h}")
                    nc.tensor.matmul(pl[:], lw9t[:, h * 128:(h + 1) * 128], vt2[:],
                                     start=True, stop=True)
                    nc.vector.scalar_tensor_tensor(hTp[:, h, :], pl[:], lb(h),
                                                   xf[:, h, cs:cs + NCHUNK],
                                                   OP.add, OP.add)
                    if h % 2 == 1:
                        yield
                h1p = ph1.tile([128, AC, NCHUNK], f8, tag="h1p", name=f"h1p_{nm}")
                for a in range(AC):
                    p1 = psmm.tile([128, NCHUNK], f32, tag="mm", name=f"p1_{nm}_{a}")
                    for q in range(KP1):
                        nc.tensor.matmul(p1[:],
                                         w1t[:, 2 * q:2 * q + 2, a * 128:(a + 1) * 128],
                                         hTp[:, 2 * q:2 * q + 2, :],
                                         start=(q == 0), stop=(q == KP1 - 1),
                                         perf_mode=DR)
                    nc.scalar.activation(h1p[:, a, :], p1[:], AF.Gelu,
                                         bias=b1(a), scale=1.0 / S1)
                    yield
                for h in range(HC):
                    p2 = psmm.tile([128, NCHUNK], f32, tag="mm", name=f"p2_{nm}_{h}")
                    for q in range(KP2):
                        nc.tensor.matmul(p2[:],
                                         w2t[:, 2 * q:2 * q + 2, h * 128:(h + 1) * 128],
                                         h1p[:, 2 * q:2 * q + 2, :],
                                         start=(q == 0), stop=(q == KP2 - 1),
                                         perf_mode=DR)
                    nc.scalar.activation(t["of"][:, h, cs:cs + NCHUNK], p2[:],
                                         AF.Gelu, bias=b2(h), scale=1.0 / S2)
                    yield

            def drain(gen):
                for _ in gen:
                    pass

            # ---- pipelined schedule: routing(c+1) interleaves into big(c) --
            import contextlib
            loop_cm = (tc.For_i(0, loop_repeat, 1) if loop_repeat > 1
                       else contextlib.nullcontext())
            if weights_outside:
                nc.sync.dma_start(w1t[:], dw1[:, :, :])
                nc.sync.dma_start(w2t[:], dw2[:, :, :])
            with loop_cm:
                for rr in range(repeat):
                    # per-iteration streaming DMAs, routing-critical first;
                    # all large-descriptor (2-12KB contiguous runs).
                    p54f = pp54.tile([NP, NCT], f32, tag="p54", name=f"p54_{rr}")
                    nc.sync.dma_start(p54f[:], dp54[:, :])
                    o0f = po0.tile([NV, NCT], f32r, tag="o0", name=f"o0_{rr}")
                    nc.sync.dma_start(o0f[:], do0[:, :])
                    xf = px.tile([128, HC, NCT], bf16, tag="xf", name=f"xf_{rr}")
                    if not no_io_dma:
                        for k in range(HC):
                            nc.sync.dma_start(xf[:, k, :], dx[k, :, :])
                    of = pout.tile([128, HC, NCT], bf16, tag="of", name=f"of_{rr}")
                    t = {"p54f": p54f, "o0f": o0f, "xf": xf, "of": of}
                    drain(routing_units(f"{rr}_0", 0, t))
                    if rr == 0 and not weights_outside:
                        # weight DMAs issued after the routing-critical DMAs
                        nc.sync.dma_start(w1t[:], dw1[:, :, :])
                        nc.sync.dma_start(w2t[:], dw2[:, :, :])
                    for c0 in range(NCH):
                        nm = f"{rr}_{c0}"
                        rgen = (routing_units(f"{rr}_{c0 + 1}", c0 + 1, t)
                                if c0 + 1 < NCH else None)
                        for _ in big_units(nm, c0, t):
                            if rgen is not None:
                                next(rgen, None)
                        if rgen is not None:
                            drain(rgen)
                    if not no_io_dma:
                        for k in range(HC):
                            nc.sync.dma_start(dout[k, :, :], of[:, k, :])

    nc.compile()
    return nc


def _sigmoid(v):
    return 1.0 / (1.0 + np.exp(-v.astype(np.float64)))


def _prep_inputs(x, t, s, fc1_w, fc1_b, fc2_w, fc2_b, efc1, efc2,
                 sem_w, sem_b, route_weights, larger_w, larger_b, elarger):
    import ml_dtypes
    f8np = ml_dtypes.float8_e4m3
    bf16np = ml_dtypes.bfloat16

    t = int(np.asarray(t).item())
    sv = float(np.asarray(s).reshape(-1)[0])
    Teff = t + 1
    NL = M3 * Teff
    NP = M3 * Teff * C

    f = np.float32
    gfc1 = _sigmoid(sv * np.asarray(efc1)[t]).astype(f)          # [A]
    gfc2 = _sigmoid(sv * np.asarray(efc2)[t]).astype(f)          # [H]
    glarger = _sigmoid(sv * np.asarray(elarger)[t]).astype(f)    # [H]

    w1T = np.zeros((H, APAD), f)
    w1T[:, :A] = np.asarray(fc1_w, f).T
    w1p = np.ascontiguousarray(
        (w1T * S1).reshape(HC, 128, APAD).transpose(1, 0, 2)).astype(f8np)
    w2g = np.zeros((APAD, H), f)
    w2g[:A] = np.asarray(fc2_w, f).T * gfc1[:, None]
    w2p = np.ascontiguousarray(
        (w2g * S2).reshape(AC, 128, H).transpose(1, 0, 2)).astype(f8np)
    lw9 = np.ascontiguousarray((np.asarray(larger_w, f) * glarger[:, None]).T)  # [9, H]
    lb = (np.asarray(larger_b, f) * glarger).astype(f)           # [H]

    b1p = np.zeros(APAD, f)
    b1p[:A] = np.asarray(fc1_b, f)
    consts = np.zeros((128, 35), f)
    consts[:, 0:16] = b1p.reshape(16, 128).T
    consts[:, 16:22] = np.asarray(fc2_b, f).reshape(6, 128).T
    consts[:, 22:28] = gfc2.reshape(6, 128).T
    consts[:, 28:34] = lb.reshape(6, 128).T
    consts[:, 34] = EPS

    # ---- host: semantic capsules -> squash -> priors (exact, f64) --------
    x2 = np.asarray(x, f).reshape(NTOK, H).astype(np.float64)
    semw = np.asarray(sem_w, np.float64).transpose(2, 1, 0).reshape(H, C * T)
    semb = np.asarray(sem_b, np.float64).T.reshape(C * T)
    sem = x2 @ semw + semb                                       # [N, 30] (c*T+t)
    g = sem.reshape(NTOK, C, T)
    sq = np.sum(g * g, axis=-1, keepdims=True) + EPS
    v = (sq / (1.0 + sq)) * g / np.sqrt(sq)                      # squash over t
    x5 = v.reshape(NTOK, T, C)
    rw = np.asarray(route_weights, np.float64)
    pri = np.einsum("nrc,mrcd->mnrd", x5[:, :Teff], rw[:, :Teff])  # [3,N,Teff,3]
    v0f = pri.mean(axis=2)                                       # [3, N, 3]
    # iter-0 squash done on host: outputs0 = squash(vote0), vote0 = v0f
    sq0 = np.sum(v0f * v0f, axis=-1, keepdims=True) + EPS
    o0f = (sq0 / (1.0 + sq0)) * v0f / np.sqrt(sq0)               # [3, N, 3]

    # selector matrices (lhsT layout [K, M])
    sq9to3 = np.zeros((NV, M3), f)
    exp3to9 = np.zeros((M3, NV), f)
    for d in range(M3):
        for cc in range(C):
            sq9to3[d * C + cc, d] = 1.0
            exp3to9[d, d * C + cc] = 1.0
    exp9toNP = np.zeros((NV, NP), f)
    redNPtoNL = np.zeros((NP, NL), f)
    expNLtoNP = np.zeros((NL, NP), f)
    redNLto3 = np.zeros((NL, M3), f)
    redNPto9 = np.zeros((NP, NV), f)
    for d in range(M3):
        for r in range(Teff):
            redNLto3[d * Teff + r, d] = 1.0
            for cc in range(C):
                q = d * Teff * C + r * C + cc
                exp9toNP[d * C + cc, q] = 1.0
                redNPtoNL[q, d * Teff + r] = 1.0
                expNLtoNP[d * Teff + r, q] = 1.0
                redNPto9[q, d * C + cc] = 1.0

    const_map = {
        "w1p": w1p, "w2p": w2p, "lw9": lw9, "consts": consts,
        "sq9to3": sq9to3, "exp3to9": exp3to9, "exp9toNP": exp9toNP,
        "redNPtoNL": redNPtoNL, "expNLtoNP": expNLtoNP, "redNLto3": redNLto3,
        "redNPto9": redNPto9,
    }

    # stream-order scramble per core: consumer (ca, nl2, j=3d+c) pulls vote of
    # (m, n') with  q = ci*3*NCT + 3*(ca*512+nl2) + d;  m = q//NTOK, n' = q%NTOK
    nl2 = np.arange(NCH * NCHUNK)                                # [2048]
    dd = np.arange(M3)
    x32 = np.asarray(x, f).reshape(NTOK, H)
    in_maps = []
    for ci in range(NCORES):
        q = ci * 3 * NCT + 3 * nl2[None, :] + dd[:, None]        # [3, 2048]
        m_idx = q // NTOK
        n_idx = q % NTOK
        blk = pri[m_idx, n_idx]                                  # [3, 2048, Teff, 3]
        p54s = np.ascontiguousarray(
            blk.transpose(0, 2, 3, 1).reshape(NP, NCT)).astype(f)
        oblk = o0f[m_idx, n_idx]                                 # [3, 2048, 3]
        o0s = np.ascontiguousarray(
            oblk.transpose(0, 2, 1).reshape(NV, NCT)).astype(f)
        xT = np.ascontiguousarray(
            x32[ci * NCT:(ci + 1) * NCT].T).astype(bf16np).reshape(HC, 128, NCT)
        m = dict(const_map)
        m["xT"] = xT
        m["p54s"] = p54s
        m["o0s"] = o0s
        in_maps.append(m)
    return Teff, in_maps, (x32, gfc2)


def run_sharded(trace=False, **inputs):
    """Run on hardware; returns (full_output [B,S,H] f32, exec_time_ns|None)."""
    from concourse.bass_utils import run_bass_kernel_spmd

    Teff, in_maps, (x32, gfc2) = _prep_inputs(**inputs)
    if Teff not in _CACHE:
        _CACHE[Teff] = _build(Teff)
    nc = _CACHE[Teff]
    last_err = None
    for _attempt in range(3):
        try:
            res = run_bass_kernel_spmd(nc, in_maps, list(range(NCORES)), trace=trace)
            break
        except Exception as e:  # transient NRT/axon device errors recover on retry
            last_err = e
    else:
        raise last_err
    full = np.empty((NTOK, H), np.float32)
    for ci in range(NCORES):
        g2 = res.results[ci]["outT"].reshape(H, NCT).astype(np.float32).T
        full[ci * NCT:(ci + 1) * NCT] = x32[ci * NCT:(ci + 1) * NCT] + g2 * gfc2
    return full.reshape(B, S, H), res.exec_time_ns


def kernel(**inputs):
    out, _ = run_sharded(trace=False, **inputs)
    return out


# revision 44
# speedup vs baseline: 1.7752x; 1.7752x over previous
"""Trainium2 Bass kernel for BertAdapterCapsuleMask — fp8 DoubleRow version.

Self-contained: takes full (unsharded) numpy inputs, shards across 8
NeuronCores, runs a fused Bass/Tile kernel per core, gathers the full output.

Key semantics note: the reference's `h_caps = vote.reshape(B, S, M*C)` is an
m-major flat reinterpret, so token n's 9 capsule inputs are vote values of
tokens ~3n from a single m-block — NOT batch-local.  We handle this by
computing the cheap part (semantic capsules -> squash -> routing priors,
~0.5% of FLOPs) exactly on the host, pre-scrambling priors into each core's
consumer "stream order" (rows (d, r, c), d = which-of-3-source-tokens), and
running the iterative routing + all heavy matmuls on device.  In stream
order the final vote tile IS h_caps in consumer layout, so the larger/adapter
matmuls consume it directly.

Perf design: the two adapter matmuls (H->A, A->H over 2048 tokens/core)
dominate PE time.  They run in fp8e4m3 with MatmulPerfMode.DoubleRow (two
128-deep K-subtiles per instruction at 0.5 cyc/row = 4x f32r row throughput,
2x fewer PE instructions).  Weights are pre-scaled (x64 / x128) on the host
to center their tiny magnitudes in fp8 range; the scale is undone for free
in the gelu activation's input-scale.  x streams in as bf16 (only feeds the
fp8 adapter input), the device returns the pre-gate second gelu in bf16, and
the host applies the (exact) gfc2 gate and adds the f32 skip connection.
Measured end-to-end max rel err ~5e-3 (gate: 2e-2).
"""

import sys

sys.path.insert(0, "/opt/trn_rl_repo")
import numpy as np

B, S, H, A, T, C, M3 = 128, 128, 768, 2000, 10, 3, 3
NCORES = 8
NTOK = B * S                  # 16384 tokens total
NCT = NTOK // NCORES          # 2048 tokens per core
NCHUNK = 512                  # tokens per pipeline chunk (PSUM bank = 512 f32)
NCH = NCT // NCHUNK           # 4 chunks per core
APAD = 2048                   # A=2000 zero-padded to 16x128
AC = APAD // 128              # 16 a-chunks
HC = H // 128                 # 6 h-chunks
KP1 = HC // 2                 # 3 DoubleRow k-pairs for mm1 (K=H)
KP2 = AC // 2                 # 8 DoubleRow k-pairs for mm2 (K=A)
EPS = 1e-16
NV = M3 * C                   # 9 rows: (d, c)
S1 = 64.0                     # fp8 pre-scale on w1
S2 = 128.0                    # fp8 pre-scale on w2

_CACHE = {}


def _sel_shapes(Teff):
    NL = M3 * Teff
    NP = M3 * Teff * C
    return {
        "sq9to3": (NV, M3),      # sum squares of vote per d
        "exp3to9": (M3, NV),     # per-d scalar -> (d, c)
        "exp9toNP": (NV, NP),    # outputs (d,c) -> (d, r, c)
        "redNPtoNL": (NP, NL),   # sum over c: (d,r,c) -> (d,r)
        "expNLtoNP": (NL, NP),   # E (d,r) -> (d,r,c)
        "redNLto3": (NL, M3),    # sum over r: (d,r) -> d
        "redNPto9": (NP, NV),    # sum over r: (d,r,c) -> (d,c)
    }


def _build(Teff, repeat=1, loop_repeat=1, psum_mm=2, psum_rt=2,
           no_io_dma=False, weights_outside=False, no_routing=False,
           streams_outside=False):
    """Build + compile the per-core Bass program (shapes depend on Teff=t+1).

    repeat>1 unrolls the whole computation R times (timing builds only)."""
    import concourse.bacc as bacc
    import concourse.mybir as mybir
    import concourse.tile as tile

    f32 = mybir.dt.float32
    f32r = mybir.dt.float32r
    bf16 = mybir.dt.bfloat16
    f8 = mybir.dt.float8e4
    i32 = mybir.dt.int32
    DR = mybir.MatmulPerfMode.DoubleRow
    AF = mybir.ActivationFunctionType
    OP = mybir.AluOpType
    # Schraudolph exp constants: exp(x) ~= bitcast_f32(round(EXP_A*x + EXP_B)),
    # max rel err 2.98% over x in [-30, 8] (bits values are exact multiples of
    # the f32 ulp at ~1e9, so round-vs-trunc convert semantics agree).
    EXP_A, EXP_B = 12102203.0, 1064987000.0
    RSQ_B = 1597463007.0      # float-domain fast-inverse-sqrt magic

    NL = M3 * Teff
    NP = M3 * Teff * C
    sel_shapes = _sel_shapes(Teff)

    nc = bacc.Bacc("TRN2", target_bir_lowering=False, debug=False)

    dx = nc.dram_tensor("xT", [HC, 128, NCT], bf16, kind="ExternalInput").ap()
    dw1 = nc.dram_tensor("w1p", [128, HC, APAD], f8, kind="ExternalInput").ap()
    dw2 = nc.dram_tensor("w2p", [128, AC, H], f8, kind="ExternalInput").ap()
    dlw9 = nc.dram_tensor("lw9", [NV, H], f32r, kind="ExternalInput").ap()
    dp54 = nc.dram_tensor("p54s", [NP, NCT], f32, kind="ExternalInput").ap()
    do0 = nc.dram_tensor("o0s", [NV, NCT], f32r, kind="ExternalInput").ap()
    dcon = nc.dram_tensor("consts", [128, 35], f32, kind="ExternalInput").ap()
    dsel = {
        k: nc.dram_tensor(k, list(v), f32r, kind="ExternalInput").ap()
        for k, v in sel_shapes.items()
    }
    dout = nc.dram_tensor("outT", [HC, 128, NCT], bf16, kind="ExternalOutput").ap()

    with tile.TileContext(nc) as tc, \
         nc.allow_low_precision(reason="fp8/bf16 tiles feed PE matmuls by design"):
        with tc.tile_pool(name="wp", bufs=1) as wp, \
             tc.tile_pool(name="px", bufs=1) as px, \
             tc.tile_pool(name="pout", bufs=1) as pout, \
             tc.tile_pool(name="ph1", bufs=1) as ph1, \
             tc.tile_pool(name="phT", bufs=1) as phT, \
             tc.tile_pool(name="prt", bufs=5) as prt, \
             tc.tile_pool(name="pp54", bufs=1) as pp54, \
             tc.tile_pool(name="po0", bufs=1) as po0, \
             tc.tile_pool(name="pL", bufs=2) as pL, \
             tc.tile_pool(name="psmm", bufs=psum_mm, space="PSUM") as psmm, \
             tc.tile_pool(name="psrt", bufs=psum_rt, space="PSUM") as psrt:

            # ---- small constant loads (selectors, consts, lw9) ------------
            selt = {}
            for k, (pp, mm) in sel_shapes.items():
                tl = wp.tile([pp, mm], f32r, name=f"sel_{k}")
                nc.sync.dma_start(tl[:], dsel[k][:, :])
                selt[k] = tl
            cont = wp.tile([128, 35], f32, name="consts")
            nc.sync.dma_start(cont[:], dcon[:, :])
            lw9t = wp.tile([NV, H], f32r, name="lw9")
            nc.sync.dma_start(lw9t[:], dlw9[:, :])
            w1t = wp.tile([128, HC, APAD], f8, name="w1p")
            w2t = wp.tile([128, AC, H], f8, name="w2p")

            b1 = lambda a: cont[:, a:a + 1]            # noqa: E731
            b2 = lambda h: cont[:, 16 + h:17 + h]      # noqa: E731
            lb = lambda h: cont[:, 28 + h:29 + h]      # noqa: E731
            epsc = lambda n: cont[0:n, 34:35]          # noqa: E731

            def coef_chain(nm, sq_ps, ngrp, tg):
                """squash coefficient from group sum-of-squares psum [ngrp,n]:
                coef = s / ((1+s) * sqrt(s)),  s = sq+eps,  f32r tile.

                sqrt computed as exp(0.5*ln(s)) so every routing ACT op lives
                in the single {Ln, Exp} table; the batched-island schedule then
                keeps Gelu<->{Ln,Exp} table swaps to a handful per iteration."""
                lnt = prt.tile([ngrp, NCHUNK], f32, tag=tg, name=f"lnt_{nm}")
                nc.scalar.activation(lnt[:], sq_ps[:], AF.Ln, bias=epsc(ngrp))
                sqr = prt.tile([ngrp, NCHUNK], f32, tag=tg, name=f"sqr_{nm}")
                nc.scalar.activation(sqr[:], lnt[:], AF.Exp, scale=0.5)
                den = prt.tile([ngrp, NCHUNK], f32, tag=tg, name=f"den_{nm}")
                nc.vector.scalar_tensor_tensor(den[:], sq_ps[:], 1.0 + EPS, sqr[:],
                                               OP.add, OP.mult)
                rec = prt.tile([ngrp, NCHUNK], f32, tag=tg, name=f"rec_{nm}")
                nc.vector.reciprocal(rec[:], den[:])
                coef = prt.tile([ngrp, NCHUNK], f32r, tag=tg, name=f"coef_{nm}")
                nc.vector.scalar_tensor_tensor(coef[:], sq_ps[:], EPS, rec[:],
                                               OP.add, OP.mult)
                return coef

            def exp_pool(nm, L_tile, npart, tg):
                """E = exp(L) on the ACT engine ({Ln, Exp} table)."""
                E = prt.tile([npart, NCHUNK], f32r, tag=tg, name=f"E_{nm}")
                nc.scalar.activation(E[:], L_tile[:], AF.Exp)
                return E

            state = {}

            def routing_units(nm, c0, t):
                """Routing chain for chunk c0, one yield per PE-anchored unit.

                Stores vt2 (h_caps tile) in state[nm]; t holds the
                per-iteration full tiles (p54f, o0f, xf, of)."""
                cs = c0 * NCHUNK
                tg = f"rt{c0 % NCH}"
                ptg = f"ps{c0 % 3}"
                ltg = f"L{c0 % NCH}"
                p54 = t["p54f"][:, cs:cs + NCHUNK]
                o0sl = t["o0f"][:, cs:cs + NCHUNK]
                yield

                def squash9_units(snm, vote_src):
                    vv = prt.tile([NV, NCHUNK], f32r, tag=tg, name=f"vv_{snm}")
                    nc.vector.tensor_mul(vv[:], vote_src[:], vote_src[:])
                    yield
                    sqm = psrt.tile([M3, NCHUNK], f32, tag=ptg, name=f"sqm_{snm}")
                    nc.tensor.matmul(sqm[:], selt["sq9to3"][:], vv[:], start=True, stop=True)
                    coef = coef_chain(snm, sqm, M3, tg)
                    yield
                    ce9 = psrt.tile([NV, NCHUNK], f32, tag=ptg, name=f"ce9_{snm}")
                    nc.tensor.matmul(ce9[:], selt["exp3to9"][:], coef[:], start=True, stop=True)
                    outp = prt.tile([NV, NCHUNK], f32r, tag=tg, name=f"outp_{snm}")
                    nc.vector.tensor_mul(outp[:], vote_src[:], ce9[:])
                    state[f"outp_{snm}"] = outp

                def delta_units(snm, outp):
                    o54 = psrt.tile([NP, NCHUNK], f32, tag=ptg, name=f"o54_{snm}")
                    nc.tensor.matmul(o54[:], selt["exp9toNP"][:], outp, start=True, stop=True)
                    prd = prt.tile([NP, NCHUNK], f32r, tag=tg, name=f"prd_{snm}")
                    nc.vector.tensor_mul(prd[:], p54, o54[:])
                    yield
                    dl = psrt.tile([NL, NCHUNK], f32, tag=ptg, name=f"dl_{snm}")
                    nc.tensor.matmul(dl[:], selt["redNPtoNL"][:], prd[:], start=True, stop=True)
                    state[f"dl_{snm}"] = dl

                def vote_units(snm, e_tile):
                    dn = psrt.tile([M3, NCHUNK], f32, tag=ptg, name=f"dn_{snm}")
                    nc.tensor.matmul(dn[:], selt["redNLto3"][:], e_tile[:], start=True, stop=True)
                    rcd = prt.tile([M3, NCHUNK], f32r, tag=tg, name=f"rcd_{snm}")
                    nc.vector.reciprocal(rcd[:], dn[:])
                    yield
                    e54 = psrt.tile([NP, NCHUNK], f32, tag=ptg, name=f"e54_{snm}")
                    nc.tensor.matmul(e54[:], selt["expNLtoNP"][:], e_tile[:], start=True, stop=True)
                    pre = prt.tile([NP, NCHUNK], f32r, tag=tg, name=f"pre_{snm}")
                    nc.vector.tensor_mul(pre[:], p54, e54[:])
                    yield
                    vu = psrt.tile([NV, NCHUNK], f32, tag=ptg, name=f"vu_{snm}")
                    nc.tensor.matmul(vu[:], selt["redNPto9"][:], pre[:], start=True, stop=True)
                    vusb = prt.tile([NV, NCHUNK], f32, tag=tg, name=f"vusb_{snm}")
                    nc.vector.tensor_copy(vusb[:], vu[:])
                    yield
                    r9 = psrt.tile([NV, NCHUNK], f32, tag=ptg, name=f"r9_{snm}")
                    nc.tensor.matmul(r9[:], selt["exp3to9"][:], rcd[:], start=True, stop=True)
                    vt = prt.tile([NV, NCHUNK], f32r, tag=tg, name=f"vt_{snm}")
                    nc.vector.tensor_mul(vt[:], vusb[:], r9[:])
                    state[f"vt_{snm}"] = vt

                # iter 0: outputs0 = squash(mean-priors) precomputed on host
                yield from delta_units(f"{nm}_0", o0sl)
                yield
                L1 = pL.tile([NL, NCHUNK], f32, tag=ltg, name=f"L1_{nm}")
                nc.vector.tensor_copy(L1[:], state[f"dl_{nm}_0"][:])
                E1 = exp_pool(f"E1_{nm}", L1, NL, tg)
                # iter 1
                yield from vote_units(f"{nm}_1", E1)
                yield
                vt1 = state[f"vt_{nm}_1"]
                yield from squash9_units(f"{nm}_1s", vt1)
                yield
                yield from delta_units(f"{nm}_1", state[f"outp_{nm}_1s"])
                yield
                L2 = pL.tile([NL, NCHUNK], f32, tag=ltg, name=f"L2_{nm}")
                nc.vector.tensor_add(L2[:], L1[:], state[f"dl_{nm}_1"][:])
                E2 = exp_pool(f"E2_{nm}", L2, NL, tg)
                # iter 2 (final)
                yield from vote_units(f"{nm}_2", E2)
                state[f"vt2_{nm}"] = state[f"vt_{nm}_2"]

            def big_units(nm, c0, t):
                """larger + adapter matmuls for chunk c0, one yield per psum group."""
                cs = c0 * NCHUNK
                vt2 = state[f"vt2_{nm}"]
                vt2ap = vt2 if no_routing else vt2[:]
                xf = t["xf"]
                hTp = phT.tile([128, HC, NCHUNK], f8, tag="hTp", name=f"hTp_{nm}")
                for h in range(HC):
                    pl = psmm.tile([128, NCHUNK], f32, tag="mm", name=f"pl_{nm}_{h}")
                    nc.tensor.matmul(pl[:], lw9t[:, h * 128:(h + 1) * 128], vt2ap,
                                     start=True, stop=True)
                    nc.vector.scalar_tensor_tensor(hTp[:, h, :], pl[:], lb(h),
                                                   xf[:, h, cs:cs + NCHUNK],
                                                   OP.add, OP.add)
                    if h % 2 == 1:
                        yield
                h1p = ph1.tile([128, AC, NCHUNK], f8, tag="h1p", name=f"h1p_{nm}")
                for a in range(AC):
                    p1 = psmm.tile([128, NCHUNK], f32, tag="mm", name=f"p1_{nm}_{a}")
                    for q in range(KP1):
                        nc.tensor.matmul(p1[:],
                                         w1t[:, 2 * q:2 * q + 2, a * 128:(a + 1) * 128],
                                         hTp[:, 2 * q:2 * q + 2, :],
                                         start=(q == 0), stop=(q == KP1 - 1),
                                         perf_mode=DR)
                    nc.scalar.activation(h1p[:, a, :], p1[:], AF.Gelu,
                                         bias=b1(a), scale=1.0 / S1)
                    yield
                for h in range(HC):
                    p2 = psmm.tile([128, NCHUNK], f32, tag="mm", name=f"p2_{nm}_{h}")
                    for q in range(KP2):
                        nc.tensor.matmul(p2[:],
                                         w2t[:, 2 * q:2 * q + 2, h * 128:(h + 1) * 128],
                                         h1p[:, 2 * q:2 * q + 2, :],
                                         start=(q == 0), stop=(q == KP2 - 1),
                                         perf_mode=DR)
                    nc.scalar.activation(t["of"][:, h, cs:cs + NCHUNK], p2[:],
                                         AF.Gelu, bias=b2(h), scale=1.0 / S2)
                    yield

            def drain(gen):
                for _ in gen:
                    pass

            # ---- pipelined schedule: routing(c+1) interleaves into big(c) --
            import contextlib
            loop_cm = (tc.For_i(0, loop_repeat, 1) if loop_repeat > 1
                       else contextlib.nullcontext())
            if weights_outside:
                nc.sync.dma_start(w1t[:], dw1[:, :, :])
                nc.sync.dma_start(w2t[:], dw2[:, :, :])
            if streams_outside:
                p54f = pp54.tile([NP, NCT], f32, tag="p54", name="p54_o")
                nc.sync.dma_start(p54f[:], dp54[:, :])
                o0f = po0.tile([NV, NCT], f32r, tag="o0", name="o0_o")
                nc.sync.dma_start(o0f[:], do0[:, :])
                xf = px.tile([128, HC, NCT], bf16, tag="xf", name="xf_o")
                for k in range(HC):
                    nc.sync.dma_start(xf[:, k, :], dx[k, :, :])
            with loop_cm:
                for rr in range(repeat):
                    # per-iteration streaming DMAs, routing-critical first;
                    # all large-descriptor (2-12KB contiguous runs).
                    if not streams_outside:
                        p54f = pp54.tile([NP, NCT], f32, tag="p54", name=f"p54_{rr}")
                        nc.sync.dma_start(p54f[:], dp54[:, :])
                        o0f = po0.tile([NV, NCT], f32r, tag="o0", name=f"o0_{rr}")
                        nc.sync.dma_start(o0f[:], do0[:, :])
                        xf = px.tile([128, HC, NCT], bf16, tag="xf", name=f"xf_{rr}")
                        if not no_io_dma:
                            for k in range(HC):
                                nc.sync.dma_start(xf[:, k, :], dx[k, :, :])
                    of = pout.tile([128, HC, NCT], bf16, tag="of", name=f"of_{rr}")
                    t = {"p54f": p54f, "o0f": o0f, "xf": xf, "of": of}
                    if no_routing:
                        for c0 in range(NCH):
                            state[f"vt2_{rr}_{c0}"] = o0f[:, c0 * NCHUNK:
                                                          (c0 + 1) * NCHUNK]
                    else:
                        drain(routing_units(f"{rr}_0", 0, t))
                    if rr == 0 and not weights_outside:
                        # weight DMAs issued after the routing-critical DMAs
                        nc.sync.dma_start(w1t[:], dw1[:, :, :])
                        nc.sync.dma_start(w2t[:], dw2[:, :, :])
                    # chunks 1-3's routing chains run concurrently, pumped in
                    # same-stage bursts during big(0) so their ACT {Ln,Exp}
                    # ops land adjacently (one table-swap island per stage
                    # instead of one per chunk); bigs 1-3 are then pure-Gelu.
                    rgens = ([] if no_routing else
                             [routing_units(f"{rr}_{c}", c, t)
                              for c in range(1, NCH)])
                    tick = 0
                    for _ in big_units(f"{rr}_0", 0, t):
                        tick += 1
                        if rgens and tick % 2 == 0:
                            any_alive = False
                            for g in rgens:
                                if next(g, "END") != "END":
                                    any_alive = True
                            if not any_alive:
                                rgens = []
                    while rgens:
                        any_alive = False
                        for g in rgens:
                            if next(g, "END") != "END":
                                any_alive = True
                        if not any_alive:
                            rgens = []
                    for c0 in range(1, NCH):
                        drain(big_units(f"{rr}_{c0}", c0, t))
                    if not no_io_dma:
                        for k in range(HC):
                            nc.sync.dma_start(dout[k, :, :], of[:, k, :])

    nc.compile()
    return nc


def _sigmoid(v):
    return 1.0 / (1.0 + np.exp(-v.astype(np.float64)))


def _prep_inputs(x, t, s, fc1_w, fc1_b, fc2_w, fc2_b, efc1, efc2,
                 sem_w, sem_b, route_weights, larger_w, larger_b, elarger):
    import ml_dtypes
    f8np = ml_dtypes.float8_e4m3
    bf16np = ml_dtypes.bfloat16

    t = int(np.asarray(t).item())
    sv = float(np.asarray(s).reshape(-1)[0])
    Teff = t + 1
    NL = M3 * Teff
    NP = M3 * Teff * C

    f = np.float32
    gfc1 = _sigmoid(sv * np.asarray(efc1)[t]).astype(f)          # [A]
    gfc2 = _sigmoid(sv * np.asarray(efc2)[t]).astype(f)          # [H]
    glarger = _sigmoid(sv * np.asarray(elarger)[t]).astype(f)    # [H]

    w1T = np.zeros((H, APAD), f)
    w1T[:, :A] = np.asarray(fc1_w, f).T
    w1p = np.ascontiguousarray(
        (w1T * S1).reshape(HC, 128, APAD).transpose(1, 0, 2)).astype(f8np)
    w2g = np.zeros((APAD, H), f)
    w2g[:A] = np.asarray(fc2_w, f).T * gfc1[:, None]
    w2p = np.ascontiguousarray(
        (w2g * S2).reshape(AC, 128, H).transpose(1, 0, 2)).astype(f8np)
    lw9 = np.ascontiguousarray((np.asarray(larger_w, f) * glarger[:, None]).T)  # [9, H]
    lb = (np.asarray(larger_b, f) * glarger).astype(f)           # [H]

    b1p = np.zeros(APAD, f)
    b1p[:A] = np.asarray(fc1_b, f)
    consts = np.zeros((128, 35), f)
    consts[:, 0:16] = b1p.reshape(16, 128).T
    consts[:, 16:22] = np.asarray(fc2_b, f).reshape(6, 128).T
    consts[:, 22:28] = gfc2.reshape(6, 128).T
    consts[:, 28:34] = lb.reshape(6, 128).T
    consts[:, 34] = EPS

    # ---- host: semantic capsules -> squash -> priors (exact, f64) --------
    x2 = np.asarray(x, f).reshape(NTOK, H).astype(np.float64)
    semw = np.asarray(sem_w, np.float64).transpose(2, 1, 0).reshape(H, C * T)
    semb = np.asarray(sem_b, np.float64).T.reshape(C * T)
    sem = x2 @ semw + semb                                       # [N, 30] (c*T+t)
    g = sem.reshape(NTOK, C, T)
    sq = np.sum(g * g, axis=-1, keepdims=True) + EPS
    v = (sq / (1.0 + sq)) * g / np.sqrt(sq)                      # squash over t
    x5 = v.reshape(NTOK, T, C)
    rw = np.asarray(route_weights, np.float64)
    pri = np.einsum("nrc,mrcd->mnrd", x5[:, :Teff], rw[:, :Teff])  # [3,N,Teff,3]
    v0f = pri.mean(axis=2)                                       # [3, N, 3]
    # iter-0 squash done on host: outputs0 = squash(vote0), vote0 = v0f
    sq0 = np.sum(v0f * v0f, axis=-1, keepdims=True) + EPS
    o0f = (sq0 / (1.0 + sq0)) * v0f / np.sqrt(sq0)               # [3, N, 3]

    # selector matrices (lhsT layout [K, M])
    sq9to3 = np.zeros((NV, M3), f)
    exp3to9 = np.zeros((M3, NV), f)
    for d in range(M3):
        for cc in range(C):
            sq9to3[d * C + cc, d] = 1.0
            exp3to9[d, d * C + cc] = 1.0
    exp9toNP = np.zeros((NV, NP), f)
    redNPtoNL = np.zeros((NP, NL), f)
    expNLtoNP = np.zeros((NL, NP), f)
    redNLto3 = np.zeros((NL, M3), f)
    redNPto9 = np.zeros((NP, NV), f)
    for d in range(M3):
        for r in range(Teff):
            redNLto3[d * Teff + r, d] = 1.0
            for cc in range(C):
                q = d * Teff * C + r * C + cc
                exp9toNP[d * C + cc, q] = 1.0
                redNPtoNL[q, d * Teff + r] = 1.0
                expNLtoNP[d * Teff + r, q] = 1.0
                redNPto9[q, d * C + cc] = 1.0

    const_map = {
        "w1p": w1p, "w2p": w2p, "lw9": lw9, "consts": consts,
        "sq9to3": sq9to3, "exp3to9": exp3to9, "exp9toNP": exp9toNP,
        "redNPtoNL": redNPtoNL, "expNLtoNP": expNLtoNP, "redNLto3": redNLto3,
        "redNPto9": redNPto9,
    }

    # stream-order scramble per core: consumer (ca, nl2, j=3d+c) pulls vote of
    # (m, n') with  q = ci*3*NCT + 3*(ca*512+nl2) + d;  m = q//NTOK, n' = q%NTOK
    nl2 = np.arange(NCH * NCHUNK)                                # [2048]
    dd = np.arange(M3)
    x32 = np.asarray(x, f).reshape(NTOK, H)
    in_maps = []
    for ci in range(NCORES):
        q = ci * 3 * NCT + 3 * nl2[None, :] + dd[:, None]        # [3, 2048]
        m_idx = q // NTOK
        n_idx = q % NTOK
        blk = pri[m_idx, n_idx]                                  # [3, 2048, Teff, 3]
        p54s = np.ascontiguousarray(
            blk.transpose(0, 2, 3, 1).reshape(NP, NCT)).astype(f)
        oblk = o0f[m_idx, n_idx]                                 # [3, 2048, 3]
        o0s = np.ascontiguousarray(
            oblk.transpose(0, 2, 1).reshape(NV, NCT)).astype(f)
        xT = np.ascontiguousarray(
            x32[ci * NCT:(ci + 1) * NCT].T).astype(bf16np).reshape(HC, 128, NCT)
        m = dict(const_map)
        m["xT"] = xT
        m["p54s"] = p54s
        m["o0s"] = o0s
        in_maps.append(m)
    return Teff, in_maps, (x32, gfc2)


def run_sharded(trace=False, **inputs):
    """Run on hardware; returns (full_output [B,S,H] f32, exec_time_ns|None)."""
    from concourse.bass_utils import run_bass_kernel_spmd

    Teff, in_maps, (x32, gfc2) = _prep_inputs(**inputs)
    if Teff not in _CACHE:
        _CACHE[Teff] = _build(Teff)
    nc = _CACHE[Teff]
    last_err = None
    for _attempt in range(3):
        try:
            res = run_bass_kernel_spmd(nc, in_maps, list(range(NCORES)), trace=trace)
            break
        except Exception as e:  # transient NRT/axon device errors recover on retry
            last_err = e
    else:
        raise last_err
    full = np.empty((NTOK, H), np.float32)
    for ci in range(NCORES):
        g2 = res.results[ci]["outT"].reshape(H, NCT).astype(np.float32).T
        full[ci * NCT:(ci + 1) * NCT] = x32[ci * NCT:(ci + 1) * NCT] + g2 * gfc2
    return full.reshape(B, S, H), res.exec_time_ns


def kernel(**inputs):
    out, _ = run_sharded(trace=False, **inputs)
    return out


# revision 49
# speedup vs baseline: 1.7972x; 1.0124x over previous
"""Trainium2 Bass kernel for BertAdapterCapsuleMask — fp8 DoubleRow version.

Self-contained: takes full (unsharded) numpy inputs, shards across 8
NeuronCores, runs a fused Bass/Tile kernel per core, gathers the full output.

Key semantics note: the reference's `h_caps = vote.reshape(B, S, M*C)` is an
m-major flat reinterpret, so token n's 9 capsule inputs are vote values of
tokens ~3n from a single m-block — NOT batch-local.  We handle this by
computing the cheap part (semantic capsules -> squash -> routing priors,
~0.5% of FLOPs) exactly on the host, pre-scrambling priors into each core's
consumer "stream order" (rows (d, r, c), d = which-of-3-source-tokens), and
running the iterative routing + all heavy matmuls on device.  In stream
order the final vote tile IS h_caps in consumer layout, so the larger/adapter
matmuls consume it directly.

Perf design: the two adapter matmuls (H->A, A->H over 2048 tokens/core)
dominate PE time.  They run in fp8e4m3 with MatmulPerfMode.DoubleRow (two
128-deep K-subtiles per instruction at 0.5 cyc/row = 4x f32r row throughput,
2x fewer PE instructions).  Weights are pre-scaled (x64 / x128) on the host
to center their tiny magnitudes in fp8 range; the scale is undone for free
in the gelu activation's input-scale.  x streams in as bf16 (only feeds the
fp8 adapter input), the device returns the pre-gate second gelu in bf16, and
the host applies the (exact) gfc2 gate and adds the f32 skip connection.
Measured end-to-end max rel err ~5e-3 (gate: 2e-2).
"""

import sys

sys.path.insert(0, "/opt/trn_rl_repo")
import numpy as np

B, S, H, A, T, C, M3 = 128, 128, 768, 2000, 10, 3, 3
NCORES = 8
NTOK = B * S                  # 16384 tokens total
NCT = NTOK // NCORES          # 2048 tokens per core
NCHUNK = 512                  # tokens per pipeline chunk (PSUM bank = 512 f32)
NCH = NCT // NCHUNK           # 4 chunks per core
APAD = 2048                   # A=2000 zero-padded to 16x128
AC = APAD // 128              # 16 a-chunks
HC = H // 128                 # 6 h-chunks
KP1 = HC // 2                 # 3 DoubleRow k-pairs for mm1 (K=H)
KP2 = AC // 2                 # 8 DoubleRow k-pairs for mm2 (K=A)
EPS = 1e-16
NV = M3 * C                   # 9 rows: (d, c)
S1 = 64.0                     # fp8 pre-scale on w1
S2 = 128.0                    # fp8 pre-scale on w2

_CACHE = {}


def _sel_shapes(Teff):
    NL = M3 * Teff
    NP = M3 * Teff * C
    return {
        "sq9to3": (NV, M3),      # sum squares of vote per d
        "exp3to9": (M3, NV),     # per-d scalar -> (d, c)
        "exp9toNP": (NV, NP),    # outputs (d,c) -> (d, r, c)
        "redNPtoNL": (NP, NL),   # sum over c: (d,r,c) -> (d,r)
        "expNLtoNP": (NL, NP),   # E (d,r) -> (d,r,c)
        "redNLto3": (NL, M3),    # sum over r: (d,r) -> d
        "redNPto9": (NP, NV),    # sum over r: (d,r,c) -> (d,c)
    }


def _build(Teff, repeat=1, loop_repeat=1, psum_mm=2, psum_rt=2,
           no_io_dma=False, weights_outside=False, no_routing=False,
           streams_outside=False):
    """Build + compile the per-core Bass program (shapes depend on Teff=t+1).

    repeat>1 unrolls the whole computation R times (timing builds only)."""
    import concourse.bacc as bacc
    import concourse.mybir as mybir
    import concourse.tile as tile

    f32 = mybir.dt.float32
    f32r = mybir.dt.float32r
    bf16 = mybir.dt.bfloat16
    f8 = mybir.dt.float8e4
    i32 = mybir.dt.int32
    DR = mybir.MatmulPerfMode.DoubleRow
    AF = mybir.ActivationFunctionType
    OP = mybir.AluOpType
    # Schraudolph exp constants: exp(x) ~= bitcast_f32(round(EXP_A*x + EXP_B)),
    # max rel err 2.98% over x in [-30, 8] (bits values are exact multiples of
    # the f32 ulp at ~1e9, so round-vs-trunc convert semantics agree).
    EXP_A, EXP_B = 12102203.0, 1064987000.0
    RSQ_B = 1597463007.0      # float-domain fast-inverse-sqrt magic

    NL = M3 * Teff
    NP = M3 * Teff * C
    sel_shapes = _sel_shapes(Teff)

    nc = bacc.Bacc("TRN2", target_bir_lowering=False, debug=False)

    dx = nc.dram_tensor("xT", [HC, 128, NCT], bf16, kind="ExternalInput").ap()
    dw1 = nc.dram_tensor("w1p", [128, HC, APAD], f8, kind="ExternalInput").ap()
    dw2 = nc.dram_tensor("w2p", [128, AC, H], f8, kind="ExternalInput").ap()
    dlw9 = nc.dram_tensor("lw9", [NV, H], f32r, kind="ExternalInput").ap()
    dp54 = nc.dram_tensor("p54s", [NP, NCT], f32, kind="ExternalInput").ap()
    do0 = nc.dram_tensor("o0s", [NV, NCT], f32r, kind="ExternalInput").ap()
    dcon = nc.dram_tensor("consts", [128, 35], f32, kind="ExternalInput").ap()
    dsel = {
        k: nc.dram_tensor(k, list(v), f32r, kind="ExternalInput").ap()
        for k, v in sel_shapes.items()
    }
    dout = nc.dram_tensor("outT", [HC, 128, NCT], bf16, kind="ExternalOutput").ap()

    with tile.TileContext(nc) as tc, \
         nc.allow_low_precision(reason="fp8/bf16 tiles feed PE matmuls by design"):
        with tc.tile_pool(name="wp", bufs=1) as wp, \
             tc.tile_pool(name="px", bufs=1) as px, \
             tc.tile_pool(name="pout", bufs=1) as pout, \
             tc.tile_pool(name="ph1", bufs=1) as ph1, \
             tc.tile_pool(name="phT", bufs=1) as phT, \
             tc.tile_pool(name="prt", bufs=5) as prt, \
             tc.tile_pool(name="pp54", bufs=1) as pp54, \
             tc.tile_pool(name="po0", bufs=1) as po0, \
             tc.tile_pool(name="pL", bufs=2) as pL, \
             tc.tile_pool(name="psmm", bufs=psum_mm, space="PSUM") as psmm, \
             tc.tile_pool(name="psrt", bufs=psum_rt, space="PSUM") as psrt:

            # ---- small constant loads (selectors, consts, lw9) ------------
            selt = {}
            for k, (pp, mm) in sel_shapes.items():
                tl = wp.tile([pp, mm], f32r, name=f"sel_{k}")
                nc.sync.dma_start(tl[:], dsel[k][:, :])
                selt[k] = tl
            cont = wp.tile([128, 35], f32, name="consts")
            nc.sync.dma_start(cont[:], dcon[:, :])
            lw9t = wp.tile([NV, H], f32r, name="lw9")
            nc.sync.dma_start(lw9t[:], dlw9[:, :])
            w1t = wp.tile([128, HC, APAD], f8, name="w1p")
            w2t = wp.tile([128, AC, H], f8, name="w2p")

            b1 = lambda a: cont[:, a:a + 1]            # noqa: E731
            b2 = lambda h: cont[:, 16 + h:17 + h]      # noqa: E731
            lb = lambda h: cont[:, 28 + h:29 + h]      # noqa: E731
            epsc = lambda n: cont[0:n, 34:35]          # noqa: E731

            def coef_chain(nm, sq_ps, ngrp, tg):
                """squash coefficient from group sum-of-squares psum [ngrp,n]:
                coef = s / ((1+s) * sqrt(s)),  s = sq+eps,  f32r tile.

                sqrt computed as exp(0.5*ln(s)) so every routing ACT op lives
                in the single {Ln, Exp} table; the batched-island schedule then
                keeps Gelu<->{Ln,Exp} table swaps to a handful per iteration."""
                lnt = prt.tile([ngrp, NCHUNK], f32, tag=tg, name=f"lnt_{nm}")
                nc.scalar.activation(lnt[:], sq_ps[:], AF.Ln, bias=epsc(ngrp))
                sqr = prt.tile([ngrp, NCHUNK], f32, tag=tg, name=f"sqr_{nm}")
                nc.scalar.activation(sqr[:], lnt[:], AF.Exp, scale=0.5)
                den = prt.tile([ngrp, NCHUNK], f32, tag=tg, name=f"den_{nm}")
                nc.vector.scalar_tensor_tensor(den[:], sq_ps[:], 1.0 + EPS, sqr[:],
                                               OP.add, OP.mult)
                rec = prt.tile([ngrp, NCHUNK], f32, tag=tg, name=f"rec_{nm}")
                nc.vector.reciprocal(rec[:], den[:])
                coef = prt.tile([ngrp, NCHUNK], f32r, tag=tg, name=f"coef_{nm}")
                nc.vector.scalar_tensor_tensor(coef[:], sq_ps[:], EPS, rec[:],
                                               OP.add, OP.mult)
                return coef

            def exp_pool(nm, L_tile, npart, tg):
                """E = exp(L) on the ACT engine ({Ln, Exp} table)."""
                E = prt.tile([npart, NCHUNK], f32r, tag=tg, name=f"E_{nm}")
                nc.scalar.activation(E[:], L_tile[:], AF.Exp)
                return E

            state = {}

            def routing_units(nm, c0, t):
                """Routing chain for chunk c0, one yield per PE-anchored unit.

                Stores vt2 (h_caps tile) in state[nm]; t holds the
                per-iteration full tiles (p54f, o0f, xf, of)."""
                cs = c0 * NCHUNK
                tg = f"rt{c0 % 2}"
                ptg = f"ps{c0 % 2}"
                ltg = f"L{c0 % 2}"
                p54 = t["p54f"][:, cs:cs + NCHUNK]
                o0sl = t["o0f"][:, cs:cs + NCHUNK]
                yield

                def squash9_units(snm, vote_src):
                    vv = prt.tile([NV, NCHUNK], f32r, tag=tg, name=f"vv_{snm}")
                    nc.vector.tensor_mul(vv[:], vote_src[:], vote_src[:])
                    yield
                    sqm = psrt.tile([M3, NCHUNK], f32, tag=ptg, name=f"sqm_{snm}")
                    nc.tensor.matmul(sqm[:], selt["sq9to3"][:], vv[:], start=True, stop=True)
                    coef = coef_chain(snm, sqm, M3, tg)
                    yield
                    ce9 = psrt.tile([NV, NCHUNK], f32, tag=ptg, name=f"ce9_{snm}")
                    nc.tensor.matmul(ce9[:], selt["exp3to9"][:], coef[:], start=True, stop=True)
                    outp = prt.tile([NV, NCHUNK], f32r, tag=tg, name=f"outp_{snm}")
                    nc.vector.tensor_mul(outp[:], vote_src[:], ce9[:])
                    state[f"outp_{snm}"] = outp

                def delta_units(snm, outp):
                    o54 = psrt.tile([NP, NCHUNK], f32, tag=ptg, name=f"o54_{snm}")
                    nc.tensor.matmul(o54[:], selt["exp9toNP"][:], outp, start=True, stop=True)
                    prd = prt.tile([NP, NCHUNK], f32r, tag=tg, name=f"prd_{snm}")
                    nc.vector.tensor_mul(prd[:], p54, o54[:])
                    yield
                    dl = psrt.tile([NL, NCHUNK], f32, tag=ptg, name=f"dl_{snm}")
                    nc.tensor.matmul(dl[:], selt["redNPtoNL"][:], prd[:], start=True, stop=True)
                    state[f"dl_{snm}"] = dl

                def vote_units(snm, e_tile):
                    dn = psrt.tile([M3, NCHUNK], f32, tag=ptg, name=f"dn_{snm}")
                    nc.tensor.matmul(dn[:], selt["redNLto3"][:], e_tile[:], start=True, stop=True)
                    rcd = prt.tile([M3, NCHUNK], f32r, tag=tg, name=f"rcd_{snm}")
                    nc.vector.reciprocal(rcd[:], dn[:])
                    yield
                    e54 = psrt.tile([NP, NCHUNK], f32, tag=ptg, name=f"e54_{snm}")
                    nc.tensor.matmul(e54[:], selt["expNLtoNP"][:], e_tile[:], start=True, stop=True)
                    pre = prt.tile([NP, NCHUNK], f32r, tag=tg, name=f"pre_{snm}")
                    nc.vector.tensor_mul(pre[:], p54, e54[:])
                    yield
                    vu = psrt.tile([NV, NCHUNK], f32, tag=ptg, name=f"vu_{snm}")
                    nc.tensor.matmul(vu[:], selt["redNPto9"][:], pre[:], start=True, stop=True)
                    vusb = prt.tile([NV, NCHUNK], f32, tag=tg, name=f"vusb_{snm}")
                    nc.vector.tensor_copy(vusb[:], vu[:])
                    yield
                    r9 = psrt.tile([NV, NCHUNK], f32, tag=ptg, name=f"r9_{snm}")
                    nc.tensor.matmul(r9[:], selt["exp3to9"][:], rcd[:], start=True, stop=True)
                    vt = prt.tile([NV, NCHUNK], f32r, tag=tg, name=f"vt_{snm}")
                    nc.vector.tensor_mul(vt[:], vusb[:], r9[:])
                    state[f"vt_{snm}"] = vt

                # iter 0: outputs0 = squash(mean-priors) precomputed on host
                yield from delta_units(f"{nm}_0", o0sl)
                yield
                L1 = pL.tile([NL, NCHUNK], f32, tag=ltg, name=f"L1_{nm}")
                nc.vector.tensor_copy(L1[:], state[f"dl_{nm}_0"][:])
                E1 = exp_pool(f"E1_{nm}", L1, NL, tg)
                # iter 1
                yield from vote_units(f"{nm}_1", E1)
                yield
                vt1 = state[f"vt_{nm}_1"]
                yield from squash9_units(f"{nm}_1s", vt1)
                yield
                yield from delta_units(f"{nm}_1", state[f"outp_{nm}_1s"])
                yield
                L2 = pL.tile([NL, NCHUNK], f32, tag=ltg, name=f"L2_{nm}")
                nc.vector.tensor_add(L2[:], L1[:], state[f"dl_{nm}_1"][:])
                E2 = exp_pool(f"E2_{nm}", L2, NL, tg)
                # iter 2 (final)
                yield from vote_units(f"{nm}_2", E2)
                state[f"vt2_{nm}"] = state[f"vt_{nm}_2"]

            def big_pair(nm, cp, t, pfx):
                """larger + adapter matmuls for chunk pair (2cp, 2cp+1): 2-bank
                [128,1024] psum tiles so each gelu / hT epilogue instruction
                covers 1024 tokens — half the ACT/DVE instruction count."""
                W = 2 * NCHUNK
                cs = cp * W
                vt2a = state[f"vt2_{pfx}_{2 * cp}"]
                vt2b = state[f"vt2_{pfx}_{2 * cp + 1}"]
                if no_routing:
                    vta, vtb = vt2a, vt2b
                else:
                    vta, vtb = vt2a[:], vt2b[:]
                xf = t["xf"]
                hTp = phT.tile([128, HC, W], f8, tag="hTp", name=f"hTp_{nm}")
                for h in range(HC):
                    pl = psmm.tile([128, W], f32, tag="mm", name=f"pl_{nm}_{h}")
                    nc.tensor.matmul(pl[:, 0:NCHUNK], lw9t[:, h * 128:(h + 1) * 128],
                                     vta, start=True, stop=True)
                    nc.tensor.matmul(pl[:, NCHUNK:W], lw9t[:, h * 128:(h + 1) * 128],
                                     vtb, start=True, stop=True)
                    nc.vector.scalar_tensor_tensor(hTp[:, h, :], pl[:], lb(h),
                                                   xf[:, h, cs:cs + W],
                                                   OP.add, OP.add)
                    if h % 2 == 1:
                        yield
                h1p = ph1.tile([128, AC, W], f8, tag="h1p", name=f"h1p_{nm}")
                for a in range(AC):
                    p1 = psmm.tile([128, W], f32, tag="mm", name=f"p1_{nm}_{a}")
                    for half in range(2):
                        sl = slice(half * NCHUNK, (half + 1) * NCHUNK)
                        for q in range(KP1):
                            nc.tensor.matmul(p1[:, sl],
                                             w1t[:, 2 * q:2 * q + 2,
                                                 a * 128:(a + 1) * 128],
                                             hTp[:, 2 * q:2 * q + 2, sl],
                                             start=(q == 0), stop=(q == KP1 - 1),
                                             perf_mode=DR)
                    nc.scalar.activation(h1p[:, a, :], p1[:], AF.Gelu,
                                         bias=b1(a), scale=1.0 / S1)
                    yield
                for h in range(HC):
                    p2 = psmm.tile([128, W], f32, tag="mm", name=f"p2_{nm}_{h}")
                    for half in range(2):
                        sl = slice(half * NCHUNK, (half + 1) * NCHUNK)
                        for q in range(KP2):
                            nc.tensor.matmul(p2[:, sl],
                                             w2t[:, 2 * q:2 * q + 2,
                                                 h * 128:(h + 1) * 128],
                                             h1p[:, 2 * q:2 * q + 2, sl],
                                             start=(q == 0), stop=(q == KP2 - 1),
                                             perf_mode=DR)
                    nc.scalar.activation(t["of"][:, h, cs:cs + W], p2[:],
                                         AF.Gelu, bias=b2(h), scale=1.0 / S2)
                    yield

            def drain(gen):
                for _ in gen:
                    pass

            # ---- pipelined schedule: routing(c+1) interleaves into big(c) --
            import contextlib
            loop_cm = (tc.For_i(0, loop_repeat, 1) if loop_repeat > 1
                       else contextlib.nullcontext())
            if weights_outside:
                nc.sync.dma_start(w1t[:], dw1[:, :, :])
                nc.sync.dma_start(w2t[:], dw2[:, :, :])
            if streams_outside:
                p54f = pp54.tile([NP, NCT], f32, tag="p54", name="p54_o")
                nc.sync.dma_start(p54f[:], dp54[:, :])
                o0f = po0.tile([NV, NCT], f32r, tag="o0", name="o0_o")
                nc.sync.dma_start(o0f[:], do0[:, :])
                xf = px.tile([128, HC, NCT], bf16, tag="xf", name="xf_o")
                for k in range(HC):
                    nc.sync.dma_start(xf[:, k, :], dx[k, :, :])
            with loop_cm:
                for rr in range(repeat):
                    # per-iteration streaming DMAs, routing-critical first;
                    # all large-descriptor (2-12KB contiguous runs).
                    if not streams_outside:
                        p54f = pp54.tile([NP, NCT], f32, tag="p54", name=f"p54_{rr}")
                        nc.sync.dma_start(p54f[:], dp54[:, :])
                        o0f = po0.tile([NV, NCT], f32r, tag="o0", name=f"o0_{rr}")
                        nc.sync.dma_start(o0f[:], do0[:, :])
                        xf = px.tile([128, HC, NCT], bf16, tag="xf", name=f"xf_{rr}")
                        if not no_io_dma:
                            for k in range(HC):
                                nc.sync.dma_start(xf[:, k, :], dx[k, :, :])
                    of = pout.tile([128, HC, NCT], bf16, tag="of", name=f"of_{rr}")
                    t = {"p54f": p54f, "o0f": o0f, "xf": xf, "of": of}
                    pfx = f"b{rr}"
                    if no_routing:
                        for c0 in range(NCH):
                            state[f"vt2_{pfx}_{c0}"] = o0f[:, c0 * NCHUNK:
                                                           (c0 + 1) * NCHUNK]
                    else:
                        # chunks 0+1's routing chains drain concurrently in
                        # stage-aligned bursts (their ACT {Ln,Exp} ops batch)
                        r01 = [routing_units(f"{pfx}_0", 0, t),
                               routing_units(f"{pfx}_1", 1, t)]
                        while r01:
                            if not any(next(g, "END") != "END" for g in r01):
                                r01 = []
                    if rr == 0 and not weights_outside:
                        # weight DMAs issued after the routing-critical DMAs
                        nc.sync.dma_start(w1t[:], dw1[:, :, :])
                        nc.sync.dma_start(w2t[:], dw2[:, :, :])
                    # chunks 2+3's routing chains pump in stage bursts during
                    # pair 0's matmul phase; pair 1 is then pure-Gelu.
                    r23 = ([] if no_routing else
                           [routing_units(f"{pfx}_2", 2, t),
                            routing_units(f"{pfx}_3", 3, t)])
                    tick = 0
                    for _ in big_pair(f"{rr}_p0", 0, t, pfx):
                        tick += 1
                        if r23 and tick % 2 == 0:
                            if not any(next(g, "END") != "END" for g in r23):
                                r23 = []
                    while r23:
                        if not any(next(g, "END") != "END" for g in r23):
                            r23 = []
                    drain(big_pair(f"{rr}_p1", 1, t, pfx))
                    if not no_io_dma:
                        for k in range(HC):
                            nc.sync.dma_start(dout[k, :, :], of[:, k, :])

    nc.compile()
    return nc


def _sigmoid(v):
    return 1.0 / (1.0 + np.exp(-v.astype(np.float64)))


def _prep_inputs(x, t, s, fc1_w, fc1_b, fc2_w, fc2_b, efc1, efc2,
                 sem_w, sem_b, route_weights, larger_w, larger_b, elarger):
    import ml_dtypes
    f8np = ml_dtypes.float8_e4m3
    bf16np = ml_dtypes.bfloat16

    t = int(np.asarray(t).item())
    sv = float(np.asarray(s).reshape(-1)[0])
    Teff = t + 1
    NL = M3 * Teff
    NP = M3 * Teff * C

    f = np.float32
    gfc1 = _sigmoid(sv * np.asarray(efc1)[t]).astype(f)          # [A]
    gfc2 = _sigmoid(sv * np.asarray(efc2)[t]).astype(f)          # [H]
    glarger = _sigmoid(sv * np.asarray(elarger)[t]).astype(f)    # [H]

    w1T = np.zeros((H, APAD), f)
    w1T[:, :A] = np.asarray(fc1_w, f).T
    w1p = np.ascontiguousarray(
        (w1T * S1).reshape(HC, 128, APAD).transpose(1, 0, 2)).astype(f8np)
    w2g = np.zeros((APAD, H), f)
    w2g[:A] = np.asarray(fc2_w, f).T * gfc1[:, None]
    w2p = np.ascontiguousarray(
        (w2g * S2).reshape(AC, 128, H).transpose(1, 0, 2)).astype(f8np)
    lw9 = np.ascontiguousarray((np.asarray(larger_w, f) * glarger[:, None]).T)  # [9, H]
    lb = (np.asarray(larger_b, f) * glarger).astype(f)           # [H]

    b1p = np.zeros(APAD, f)
    b1p[:A] = np.asarray(fc1_b, f)
    consts = np.zeros((128, 35), f)
    consts[:, 0:16] = b1p.reshape(16, 128).T
    consts[:, 16:22] = np.asarray(fc2_b, f).reshape(6, 128).T
    consts[:, 22:28] = gfc2.reshape(6, 128).T
    consts[:, 28:34] = lb.reshape(6, 128).T
    consts[:, 34] = EPS

    # ---- host: semantic capsules -> squash -> priors (exact, f64) --------
    x2 = np.asarray(x, f).reshape(NTOK, H).astype(np.float64)
    semw = np.asarray(sem_w, np.float64).transpose(2, 1, 0).reshape(H, C * T)
    semb = np.asarray(sem_b, np.float64).T.reshape(C * T)
    sem = x2 @ semw + semb                                       # [N, 30] (c*T+t)
    g = sem.reshape(NTOK, C, T)
    sq = np.sum(g * g, axis=-1, keepdims=True) + EPS
    v = (sq / (1.0 + sq)) * g / np.sqrt(sq)                      # squash over t
    x5 = v.reshape(NTOK, T, C)
    rw = np.asarray(route_weights, np.float64)
    pri = np.einsum("nrc,mrcd->mnrd", x5[:, :Teff], rw[:, :Teff])  # [3,N,Teff,3]
    v0f = pri.mean(axis=2)                                       # [3, N, 3]
    # iter-0 squash done on host: outputs0 = squash(vote0), vote0 = v0f
    sq0 = np.sum(v0f * v0f, axis=-1, keepdims=True) + EPS
    o0f = (sq0 / (1.0 + sq0)) * v0f / np.sqrt(sq0)               # [3, N, 3]

    # selector matrices (lhsT layout [K, M])
    sq9to3 = np.zeros((NV, M3), f)
    exp3to9 = np.zeros((M3, NV), f)
    for d in range(M3):
        for cc in range(C):
            sq9to3[d * C + cc, d] = 1.0
            exp3to9[d, d * C + cc] = 1.0
    exp9toNP = np.zeros((NV, NP), f)
    redNPtoNL = np.zeros((NP, NL), f)
    expNLtoNP = np.zeros((NL, NP), f)
    redNLto3 = np.zeros((NL, M3), f)
    redNPto9 = np.zeros((NP, NV), f)
    for d in range(M3):
        for r in range(Teff):
            redNLto3[d * Teff + r, d] = 1.0
            for cc in range(C):
                q = d * Teff * C + r * C + cc
                exp9toNP[d * C + cc, q] = 1.0
                redNPtoNL[q, d * Teff + r] = 1.0
                expNLtoNP[d * Teff + r, q] = 1.0
                redNPto9[q, d * C + cc] = 1.0

    const_map = {
        "w1p": w1p, "w2p": w2p, "lw9": lw9, "consts": consts,
        "sq9to3": sq9to3, "exp3to9": exp3to9, "exp9toNP": exp9toNP,
        "redNPtoNL": redNPtoNL, "expNLtoNP": expNLtoNP, "redNLto3": redNLto3,
        "redNPto9": redNPto9,
    }

    # stream-order scramble per core: consumer (ca, nl2, j=3d+c) pulls vote of
    # (m, n') with  q = ci*3*NCT + 3*(ca*512+nl2) + d;  m = q//NTOK, n' = q%NTOK
    nl2 = np.arange(NCH * NCHUNK)                                # [2048]
    dd = np.arange(M3)
    x32 = np.asarray(x, f).reshape(NTOK, H)
    in_maps = []
    for ci in range(NCORES):
        q = ci * 3 * NCT + 3 * nl2[None, :] + dd[:, None]        # [3, 2048]
        m_idx = q // NTOK
        n_idx = q % NTOK
        blk = pri[m_idx, n_idx]                                  # [3, 2048, Teff, 3]
        p54s = np.ascontiguousarray(
            blk.transpose(0, 2, 3, 1).reshape(NP, NCT)).astype(f)
        oblk = o0f[m_idx, n_idx]                                 # [3, 2048, 3]
        o0s = np.ascontiguousarray(
            oblk.transpose(0, 2, 1).reshape(NV, NCT)).astype(f)
        xT = np.ascontiguousarray(
            x32[ci * NCT:(ci + 1) * NCT].T).astype(bf16np).reshape(HC, 128, NCT)
        m = dict(const_map)
        m["xT"] = xT
        m["p54s"] = p54s
        m["o0s"] = o0s
        in_maps.append(m)
    return Teff, in_maps, (x32, gfc2)


def run_sharded(trace=False, **inputs):
    """Run on hardware; returns (full_output [B,S,H] f32, exec_time_ns|None)."""
    from concourse.bass_utils import run_bass_kernel_spmd

    Teff, in_maps, (x32, gfc2) = _prep_inputs(**inputs)
    if Teff not in _CACHE:
        _CACHE[Teff] = _build(Teff)
    nc = _CACHE[Teff]
    last_err = None
    for _attempt in range(3):
        try:
            res = run_bass_kernel_spmd(nc, in_maps, list(range(NCORES)), trace=trace)
            break
        except Exception as e:  # transient NRT/axon device errors recover on retry
            last_err = e
    else:
        raise last_err
    full = np.empty((NTOK, H), np.float32)
    for ci in range(NCORES):
        g2 = res.results[ci]["outT"].reshape(H, NCT).astype(np.float32).T
        full[ci * NCT:(ci + 1) * NCT] = x32[ci * NCT:(ci + 1) * NCT] + g2 * gfc2
    return full.reshape(B, S, H), res.exec_time_ns


def kernel(**inputs):
    out, _ = run_sharded(trace=False, **inputs)
    return out
